# revision 16
# baseline (speedup 1.0000x reference)
"""EGNN (3-layer EGNNConv + global_add_pool + MLP head) on 8 TRN2 NeuronCores.

Sharding: nodes are split into 8 contiguous shards (dst-owner sharding);
edges are sorted by dst on the host, assigned to the owning core, and
grouped by 128-node destination tiles. Per 512-edge block, feat/coord[src]
come from an indirect-DMA gather out of a replicated DRAM node table,
feat/coord[dst] from a one-hot matmul against the core-local shard tile,
and the segment-sum scatter is a one-hot matmul accumulated in PSUM.
Updated node features are all-gathered between layers. The device emits
per-core global_add_pool partials [64, 99]; the tiny head MLP runs on host.
"""
from contextlib import ExitStack

import numpy as np

import concourse.bass as bass
from concourse import bacc, mybir
from concourse.bass_utils import run_bass_kernel_spmd
from concourse.masks import make_identity
from concourse.tile import TileContext

F32 = mybir.dt.float32
I32 = mybir.dt.int32
AF = mybir.ActivationFunctionType
OP = mybir.AluOpType

NCORES = 8
P = 128
D = 100          # node-table row: feat 0:96, coord 96:99, pad 99
FPAD = 96        # padded node-feature width (= OUT)
HID = 192
OUT = 96
POS = 3
EF = 16
G = 64
KE1 = 2 * FPAD + 1 + EF   # 209 edge-MLP input rows


def _npf(x):
    return np.ascontiguousarray(np.asarray(x), dtype=np.float32)


# ----------------------------------------------------------------- host prep
def _prep(feat, coord, edge_attr, edge_index, batch, params):
    N = feat.shape[0]
    F0 = feat.shape[1]
    NSH = -(-N // NCORES)
    NT = -(-NSH // P)
    NSHP = NT * P
    NTAB = NCORES * NSHP

    src = np.asarray(edge_index[0]).astype(np.int64)
    dst = np.asarray(edge_index[1]).astype(np.int64)
    ea = _npf(edge_attr)
    order = np.argsort(dst, kind='stable')
    src_s, dst_s, ea_s = src[order], dst[order], ea[order]

    cb = np.searchsorted(dst_s, np.arange(NCORES + 1) * NSH)
    counts = np.zeros((NCORES, NT), np.int64)
    percore = []
    for c in range(NCORES):
        sl = slice(cb[c], cb[c + 1])
        drel = dst_s[sl] - c * NSH
        tl = drel // P
        counts[c] = np.bincount(tl, minlength=NT)
        percore.append((src_s[sl], drel, ea_s[sl], tl))

    chunks_t = np.maximum(1, -(-counts.max(0) // P)).astype(np.int64)
    chunk_base = np.concatenate([[0], np.cumsum(chunks_t)])
    CH = int(chunk_base[-1])
    EPAD = CH * P

    blocks = []   # (tile, nch, ch0)
    for t in range(NT):
        rem, ch0 = int(chunks_t[t]), int(chunk_base[t])
        while rem > 0:
            nch = min(4, rem)
            blocks.append((t, nch, ch0))
            ch0 += nch
            rem -= nch

    batch_np = np.asarray(batch).astype(np.int64)

    T0 = np.zeros((NTAB, D), np.float32)
    fe = _npf(feat)
    co = _npf(coord)
    for c in range(NCORES):
        n0, n1 = c * NSH, min((c + 1) * NSH, N)
        r0 = c * NSHP
        T0[r0:r0 + (n1 - n0), 0:F0] = fe[n0:n1]
        T0[r0:r0 + (n1 - n0), FPAD:FPAD + POS] = co[n0:n1]

    core_inputs = []
    for c in range(NCORES):
        ss, drel, eas, tl = percore[c]
        tf = np.searchsorted(tl, np.arange(NT))
        pos = np.arange(len(tl))
        rank = pos - tf[tl]
        dsti = chunk_base[tl] * P + rank
        srcg = np.zeros(EPAD, np.int32)
        srcg[dsti] = ((ss // NSH) * NSHP + (ss % NSH)).astype(np.int32)
        drelf = np.full(EPAD, -1.0, np.float32)
        drelf[dsti] = (drel - tl * P).astype(np.float32)
        eaT = np.zeros((EF + 1, EPAD), np.float32)
        eaT[0:EF, dsti] = eas.T
        eaT[EF, :] = 1.0
        slots = np.arange(NSHP)
        node = np.minimum(c * NSH + slots, N - 1)
        br = np.where((slots < NSH) & (c * NSH + slots < N),
                      batch_np[node], 999).astype(np.float32)
        core_inputs.append({
            'srcI': np.ascontiguousarray(srcg.reshape(CH, P).T),
            'dstR': np.ascontiguousarray(drelf.reshape(CH, P).T),
            'eaT': eaT,
            'batchR': np.ascontiguousarray(br.reshape(NT, P).T),
            'own_tab': np.ascontiguousarray(T0[c * NSHP:(c + 1) * NSHP]),
        })

    layers = []
    in_sizes = [F0, OUT, OUT]
    for li, lp in enumerate(params['layers']):
        fi = in_sizes[li]
        w1, b1 = map(_npf, lp['ew1'])
        w2, b2 = map(_npf, lp['ew2'])
        cc1, c1b = map(_npf, lp['cw1'])
        cc2 = _npf(lp['cw2'][0])
        nn1, n1b = map(_npf, lp['nw1'])
        nn2, n2b = map(_npf, lp['nw2'])
        W1p = np.zeros((KE1, HID), np.float32)
        W1p[0:fi] = w1[0:fi]
        W1p[FPAD:FPAD + fi] = w1[fi:2 * fi]
        W1p[2 * FPAD] = w1[2 * fi]
        W1p[2 * FPAD + 1:KE1] = w1[2 * fi + 1:]
        NW1p = np.zeros((FPAD + HID, HID), np.float32)
        NW1p[0:fi] = nn1[0:fi]
        NW1p[FPAD:FPAD + HID] = nn1[fi:fi + HID]
        layers.append({
            f'ew1k0_{li}': W1p[0:P],
            f'ew1k1_{li}': np.vstack([W1p[P:KE1], b1[None, :]]),      # [82,192]
            f'ew2k0_{li}': np.vstack([w2[0:FPAD], b2[None, :]]),      # [97,192]
            f'ew2k1_{li}': w2[FPAD:HID],                              # [96,192]
            f'cw1k0_{li}': np.vstack([cc1[0:FPAD], c1b[None, :]]),
            f'cw1k1_{li}': cc1[FPAD:HID],
            f'cw2k0_{li}': cc2[0:FPAD],                               # [96,1]
            f'cw2k1_{li}': cc2[FPAD:HID],
            f'nw1k0_{li}': np.vstack([NW1p[0:FPAD], n1b[None, :]]),   # [97,192]
            f'nw1k1_{li}': NW1p[FPAD:FPAD + P],                       # [128,192]
            f'nw1k2_{li}': NW1p[FPAD + P:FPAD + HID],                 # [64,192]
            f'nw2k0_{li}': np.vstack([nn2[0:FPAD], n2b[None, :]]),    # [97,96]
            f'nw2k1_{li}': nn2[FPAD:HID],                             # [96,96]
        })
    wdict = {}
    for lw in layers:
        wdict.update({k: np.ascontiguousarray(v) for k, v in lw.items()})

    head_w1, head_b1 = map(_npf, params['head']['w1'])
    head_w2, head_b2 = map(_npf, params['head']['w2'])

    plan = {
        'N': N, 'F0': F0, 'NSH': NSH, 'NT': NT, 'NSHP': NSHP,
        'NTAB': NTAB, 'CH': CH, 'EPAD': EPAD, 'blocks': tuple(blocks),
    }
    shared = dict(wdict)
    shared['T0'] = T0
    head = (head_w1, head_b1, head_w2, head_b2)
    return plan, shared, core_inputs, head


# -------------------------------------------------------------- bass builder
WSHAPES = [('ew1k0', [P, HID]), ('ew1k1', [82, HID]),
           ('ew2k0', [97, HID]), ('ew2k1', [96, HID]),
           ('cw1k0', [97, HID]), ('cw1k1', [96, HID]),
           ('cw2k0', [96, 1]), ('cw2k1', [96, 1]),
           ('nw1k0', [97, HID]), ('nw1k1', [P, HID]),
           ('nw1k2', [64, HID]), ('nw2k0', [97, OUT]),
           ('nw2k1', [96, OUT])]


def _fast_rsqrt(nc, pool, r_ap, nch):
    """y = 1/sqrt(max(r,1e-30)); token-major [128, nch]; all on DVE."""
    rcl = pool.tile([P, 4], F32, tag='rs_rcl')
    nc.vector.tensor_scalar_max(out=rcl[:, 0:nch], in0=r_ap, scalar1=1e-30)
    yi = pool.tile([P, 4], I32, tag='rs_yi')
    nc.vector.tensor_scalar(out=yi[:, 0:nch], in0=rcl[:, 0:nch].bitcast(I32),
                            scalar1=1, scalar2=None, op0=OP.arith_shift_right)
    nc.vector.tensor_scalar(out=yi[:, 0:nch], in0=yi[:, 0:nch], scalar1=-1,
                            scalar2=0x5f3759df, op0=OP.mult, op1=OP.add)
    y = yi[:, 0:nch].bitcast(F32)
    h = pool.tile([P, 4], F32, tag='rs_h')
    nc.vector.tensor_scalar(out=h[:, 0:nch], in0=rcl[:, 0:nch], scalar1=0.5,
                            scalar2=None, op0=OP.mult)
    t = pool.tile([P, 4], F32, tag='rs_t')
    for _ in range(2):
        nc.vector.tensor_tensor(out=t[:, 0:nch], in0=y, in1=y, op=OP.mult)
        nc.vector.tensor_tensor(out=t[:, 0:nch], in0=t[:, 0:nch],
                                in1=h[:, 0:nch], op=OP.mult)
        nc.vector.tensor_scalar(out=t[:, 0:nch], in0=t[:, 0:nch],
                                scalar1=-1.0, scalar2=1.5, op0=OP.mult,
                                op1=OP.add)
        nc.vector.tensor_tensor(out=y, in0=y, in1=t[:, 0:nch], op=OP.mult)
    return yi


def _build(plan):
    NT, NSHP, NTAB = plan['NT'], plan['NSHP'], plan['NTAB']
    CH, EPAD = plan['CH'], plan['EPAD']
    blocks = list(plan['blocks'])

    nc = bacc.Bacc('TRN2', target_bir_lowering=False, debug=False,
                   num_devices=NCORES)

    T0 = nc.dram_tensor('T0', [NTAB, D], F32, kind='ExternalInput')
    own_tab = nc.dram_tensor('own_tab', [NSHP, D], F32, kind='ExternalInput')
    srcI = nc.dram_tensor('srcI', [P, CH], I32, kind='ExternalInput')
    dstR = nc.dram_tensor('dstR', [P, CH], F32, kind='ExternalInput')
    eaT = nc.dram_tensor('eaT', [EF + 1, EPAD], F32,
                     kind='ExternalInput')
    batchR = nc.dram_tensor('batchR', [P, NT], F32, kind='ExternalInput')
    wt = {}
    for li in range(3):
        for nm, shp in WSHAPES:
            wt[f'{nm}_{li}'] = nc.dram_tensor(f'{nm}_{li}', shp, F32,
                                              kind='ExternalInput')
    pooled = nc.dram_tensor('pooled', [G, OUT + POS], F32,
                            kind='ExternalOutput')
    import os
    dbg = os.environ.get('KERNEL_DBG') == '1'
    if dbg:
        dbg0 = nc.dram_tensor('dbg0', [NSHP, D], F32, kind='ExternalOutput')
        dbg1 = nc.dram_tensor('dbg1', [NSHP, D], F32, kind='ExternalOutput')
        dbg2 = nc.dram_tensor('dbg2', [P, 384], F32, kind='ExternalOutput')
        nc._dbg2 = dbg2
        dbg3 = nc.dram_tensor('dbg3', [P, 196], F32, kind='ExternalOutput')
        dbg4 = nc.dram_tensor('dbg4', [97, 1024], F32, kind='ExternalOutput')

    with TileContext(nc) as tc, ExitStack() as ctx:
        cp = ctx.enter_context(tc.tile_pool(name='const', bufs=1))
        sb = ctx.enter_context(tc.tile_pool(name='work', bufs=2))
        sb3 = ctx.enter_context(tc.tile_pool(name='work3', bufs=3))
        ps = ctx.enter_context(tc.tile_pool(name='psA', bufs=3, space='PSUM'))
        pa = ctx.enter_context(tc.tile_pool(name='psacc', bufs=1,
                                            space='PSUM'))
        pb = ctx.enter_context(tc.tile_pool(name='psB', bufs=2, space='PSUM'))
        dr = ctx.enter_context(tc.tile_pool(name='dram', bufs=1,
                                            space='DRAM'))

        ident = cp.tile([P, P], F32)
        make_identity(nc, ident[:])
        iota_row = cp.tile([P, P], F32)
        nc.gpsimd.iota(iota_row[:], pattern=[[1, P]], base=0,
                       channel_multiplier=0,
                       allow_small_or_imprecise_dtypes=True)
        iotaG = cp.tile([P, G], F32)
        nc.gpsimd.iota(iotaG[:], pattern=[[1, G]], base=0,
                       channel_multiplier=0,
                       allow_small_or_imprecise_dtypes=True)
        ones1x3 = cp.tile([1, POS], F32)
        nc.vector.memset(ones1x3[:], 1.0)
        sel3 = cp.tile([4, POS], F32)
        nc.gpsimd.iota(sel3[:], pattern=[[0, POS]], base=-3,
                       channel_multiplier=1,
                       allow_small_or_imprecise_dtypes=True)
        nc.vector.tensor_scalar(out=sel3[:], in0=sel3[:], scalar1=0.0,
                                scalar2=None, op0=OP.is_equal)

        src_t = cp.tile([P, CH], I32)
        nc.sync.dma_start(out=src_t[:], in_=srcI.ap())
        dst_t = cp.tile([P, CH], F32)
        nc.sync.dma_start(out=dst_t[:], in_=dstR.ap())
        batch_t = cp.tile([P, NT], F32)
        nc.sync.dma_start(out=batch_t[:], in_=batchR.ap())

        W = {}
        for li in range(3):
            for nm, shp in WSHAPES:
                key = f'{nm}_{li}'
                w = cp.tile(shp, F32, name=f'w_{key}')
                nc.sync.dma_start(out=w[:], in_=wt[key].ap())
                W[key] = w

        shard_out = [dr.tile([NSHP, D], F32, name=f'shard_out_{li}')
                     for li in range(2)]
        Tnext = [dr.tile([NTAB, D], F32, addr_space='Shared',
                         name=f'T_{li + 1}') for li in range(2)]

        pool_acc = cp.tile([G, OUT + POS], F32)
        nc.vector.memset(pool_acc[:], 0.0)

        for li in range(3):
            Tcur = T0.ap() if li == 0 else Tnext[li - 1][:]
            own_cur = own_tab.ap() if li == 0 else shard_out[li - 1][:]
            e1k0, e1k1 = W[f'ew1k0_{li}'], W[f'ew1k1_{li}']
            e2k0, e2k1 = W[f'ew2k0_{li}'], W[f'ew2k1_{li}']
            c1k0, c1k1 = W[f'cw1k0_{li}'], W[f'cw1k1_{li}']
            c2k0, c2k1 = W[f'cw2k0_{li}'], W[f'cw2k1_{li}']

            cur_tile = -1
            shard = hacc = None
            for (t, nch, ch0) in blocks + [(-1, 0, 0)]:
                if t != cur_tile:
                    if cur_tile >= 0:
                        _node_stage(nc, sb, ps, W, li, cur_tile, shard, hacc,
                                    ident, sel3, iotaG, batch_t, pool_acc,
                                    shard_out)
                    if t < 0:
                        break
                    cur_tile = t
                    shard = sb3.tile([P, D], F32, tag='shard')
                    nc.sync.dma_start(out=shard[:],
                                      in_=own_cur[t * P:(t + 1) * P, :])
                    # hacc: [:, 0:128] = feats 0:128 x nodes;
                    #       [0:68, 128:256] = feats 128:196 x nodes
                    hacc = sb3.tile([P, 384], F32, tag='hacc')
                    nc.gpsimd.memset(hacc[:], 0.0)

                NE = nch * P
                eoff = ch0 * P
                g = sb.tile([P, 4 * D], F32, tag='g')
                for c in range(nch):
                    nc.gpsimd.indirect_dma_start(
                        out=g[:, c * D:(c + 1) * D], out_offset=None,
                        in_=Tcur,
                        in_offset=bass.IndirectOffsetOnAxis(
                            ap=src_t[:, ch0 + c:ch0 + c + 1], axis=0))
                srcTp = ps.tile([D, 512], F32, tag='ps', bufs=3, padded_shape=[P, 512])
                for c in range(nch):
                    nc.tensor.transpose(out=srcTp[:, c * P:(c + 1) * P],
                                        in_=g[:, c * D:(c + 1) * D],
                                        identity=ident[:])
                en = sb.tile([P, 512], F32, tag='en')
                nc.vector.tensor_tensor(
                    out=en[:, 0:NE].rearrange('p (c e) -> p c e', c=nch),
                    in0=dst_t[:, ch0:ch0 + nch].unsqueeze(2)
                        .to_broadcast([P, nch, P]),
                    in1=iota_row[:].unsqueeze(1).to_broadcast([P, nch, P]),
                    op=OP.is_equal)
                nep = ps.tile([P, 512], F32, tag='ps', bufs=3, padded_shape=[P, 512])
                for c in range(nch):
                    nc.tensor.transpose(out=nep[:, c * P:(c + 1) * P],
                                        in_=en[:, c * P:(c + 1) * P],
                                        identity=ident[:])
                ne = sb.tile([P, 512], F32, tag='ne')
                nc.scalar.copy(ne[:, 0:NE], nep[:, 0:NE])
                dstF = ps.tile([D, 512], F32, tag='ps', bufs=3, padded_shape=[P, 512])
                nc.tensor.matmul(dstF[:, 0:NE], lhsT=shard[:],
                                 rhs=ne[:, 0:NE], start=True, stop=True)
                K0 = sb.tile([P, 512], F32, tag='K0')
                nc.scalar.copy(K0[0:FPAD, 0:NE], srcTp[0:FPAD, 0:NE])
                nc.vector.tensor_copy(K0[FPAD:P, 0:NE], dstF[0:32, 0:NE])
                K1 = sb.tile([82, 512], F32, tag='K1')
                nc.vector.tensor_copy(K1[0:32, 0:NE], dstF[32:64, 0:NE])
                nc.vector.tensor_copy(K1[32:64, 0:NE], dstF[64:FPAD, 0:NE])
                nc.sync.dma_start(out=K1[65:82, 0:NE],
                                  in_=eaT.ap()[:, eoff:eoff + NE])
                cD = sb.tile([POS, 512], F32, tag='cD')
                nc.scalar.copy(cD[:, 0:NE], dstF[FPAD:FPAD + POS, 0:NE])
                xd = sb.tile([POS, 512], F32, tag='xd')
                nc.vector.tensor_tensor(out=xd[:, 0:NE],
                                        in0=srcTp[FPAD:FPAD + POS, 0:NE],
                                        in1=cD[:, 0:NE], op=OP.subtract)
                # radial & rsqrt, token-major
                xdTp = ps.tile([P, 12], F32, tag='psT', bufs=2, padded_shape=[P, 512])
                for c in range(nch):
                    nc.tensor.transpose(out=xdTp[:, c * POS:(c + 1) * POS],
                                        in_=xd[:, c * P:(c + 1) * P],
                                        identity=ident[0:POS, 0:POS])
                xdtm = sb.tile([P, 12], F32, tag='xdtm')
                nc.vector.tensor_copy(xdtm[:, 0:nch * POS],
                                      xdTp[:, 0:nch * POS])
                sq = sb.tile([P, 12], F32, tag='sq')
                nc.vector.tensor_tensor(out=sq[:, 0:nch * POS],
                                        in0=xdtm[:, 0:nch * POS],
                                        in1=xdtm[:, 0:nch * POS], op=OP.mult)
                rtm = sb.tile([P, 4], F32, tag='rtm')
                nc.vector.reduce_sum(
                    rtm[:, 0:nch],
                    sq[:, 0:nch * POS].rearrange('p (c k) -> p c k', k=POS),
                    axis=mybir.AxisListType.X)
                radp = ps.tile([1, 512], F32, tag='psT', bufs=2, padded_shape=[P, 512])
                for c in range(nch):
                    nc.tensor.transpose(out=radp[:, c * P:(c + 1) * P],
                                        in_=rtm[:, c:c + 1],
                                        identity=ident[:])
                nc.vector.tensor_copy(K1[64:65, 0:NE], radp[:, 0:NE])
                ytile = _fast_rsqrt(nc, sb, rtm[:, 0:nch], nch)
                rnp = ps.tile([1, 512], F32, tag='psT', bufs=2, padded_shape=[P, 512])
                for c in range(nch):
                    nc.tensor.transpose(out=rnp[:, c * P:(c + 1) * P],
                                        in_=ytile[0:P, c:c + 1].bitcast(F32),
                                        identity=ident[:])
                rn = sb.tile([1, 512], F32, tag='rn')
                nc.vector.tensor_copy(rn[:, 0:NE], rnp[:, 0:NE])
                # edge MLP (fixed 0/512 offsets keep PSUM writes bank-aligned)
                h1 = sb.tile([97, 1024], F32, tag='h1')
                for m in range(2):
                    ms = slice(m * FPAD, (m + 1) * FPAD)
                    o = m * 512
                    h1p = pb.tile([FPAD, 512], F32, tag='pb', bufs=2,
                                  padded_shape=[P, 512])
                    nc.tensor.matmul(h1p[:, 0:NE], lhsT=e1k0[:, ms],
                                     rhs=K0[:, 0:NE], start=True, stop=False)
                    nc.tensor.matmul(h1p[:, 0:NE], lhsT=e1k1[:, ms],
                                     rhs=K1[:, 0:NE], start=False, stop=True)
                    nc.scalar.activation(h1[0:FPAD, o:o + NE], h1p[:, 0:NE],
                                         AF.Silu)
                nc.gpsimd.memset(h1[FPAD:97, 0:NE], 1.0)
                msg = sb.tile([97, 1024], F32, tag='msg')
                for m in range(2):
                    ms = slice(m * FPAD, (m + 1) * FPAD)
                    o = m * 512
                    msgp = pb.tile([FPAD, 512], F32, tag='pb', bufs=2,
                                   padded_shape=[P, 512])
                    nc.tensor.matmul(msgp[:, 0:NE], lhsT=e2k0[:, ms],
                                     rhs=h1[0:97, 0:NE], start=True,
                                     stop=False)
                    nc.tensor.matmul(msgp[:, 0:NE], lhsT=e2k1[:, ms],
                                     rhs=h1[0:FPAD, 512:512 + NE],
                                     start=False, stop=True)
                    nc.scalar.activation(msg[0:FPAD, o:o + NE], msgp[:, 0:NE],
                                         AF.Silu)
                nc.gpsimd.memset(msg[FPAD:97, 0:NE], 1.0)
                c1t = sb.tile([FPAD, 1024], F32, tag='c1t')
                for m in range(2):
                    ms = slice(m * FPAD, (m + 1) * FPAD)
                    o = m * 512
                    c1p = pb.tile([FPAD, 512], F32, tag='pb', bufs=2,
                                  padded_shape=[P, 512])
                    nc.tensor.matmul(c1p[:, 0:NE], lhsT=c1k0[:, ms],
                                     rhs=msg[0:97, 0:NE], start=True,
                                     stop=False)
                    nc.tensor.matmul(c1p[:, 0:NE], lhsT=c1k1[:, ms],
                                     rhs=msg[0:FPAD, 512:512 + NE],
                                     start=False, stop=True)
                    nc.scalar.activation(c1t[:, o:o + NE], c1p[:, 0:NE],
                                         AF.Silu)
                wp = ps.tile([1, 512], F32, tag='psT', bufs=2, padded_shape=[P, 512])
                nc.tensor.matmul(wp[:, 0:NE], lhsT=c2k0[:], rhs=c1t[:, 0:NE],
                                 start=True, stop=False)
                nc.tensor.matmul(wp[:, 0:NE], lhsT=c2k1[:],
                                 rhs=c1t[:, 512:512 + NE], start=False,
                                 stop=True)
                wrn = sb.tile([1, 512], F32, tag='wrn')
                nc.vector.tensor_tensor(out=wrn[:, 0:NE], in0=rn[:, 0:NE],
                                        in1=wp[:, 0:NE], op=OP.mult)
                wb3 = ps.tile([POS, 512], F32, tag='psT', bufs=2, padded_shape=[P, 512])
                nc.tensor.matmul(wb3[:, 0:NE], lhsT=ones1x3[:],
                                 rhs=wrn[:, 0:NE], start=True, stop=True)
                extras = sb.tile([4, 512], F32, tag='extras')
                nc.vector.tensor_tensor(out=extras[0:POS, 0:NE],
                                        in0=xd[:, 0:NE], in1=wb3[:, 0:NE],
                                        op=OP.mult)
                nc.sync.dma_start(out=extras[POS:4, 0:NE],
                                  in_=eaT.ap()[EF:EF + 1, eoff:eoff + NE])
                accp = pa.tile([P, 384], F32, tag='acc', bufs=1, padded_shape=[P, 512])
                for c in range(nch):
                    payT = ps.tile([P, 196], F32, tag='ps', bufs=3, padded_shape=[P, 512])
                    nc.tensor.transpose(
                        out=payT[:, 0:FPAD],
                        in_=msg[0:FPAD, c * P:(c + 1) * P],
                        identity=ident[0:FPAD, 0:FPAD])
                    nc.tensor.transpose(
                        out=payT[:, FPAD:2 * FPAD],
                        in_=msg[0:FPAD, 512 + c * P:512 + (c + 1) * P],
                        identity=ident[0:FPAD, 0:FPAD])
                    nc.tensor.transpose(
                        out=payT[:, 2 * FPAD:196],
                        in_=extras[:, c * P:(c + 1) * P],
                        identity=ident[0:4, 0:4])
                    payS = sb.tile([P, 196], F32, tag='payS')
                    nc.vector.tensor_copy(payS[:], payT[:])
                    if dbg and li == 0 and ch0 == 0 and c == 0:
                        nc.sync.dma_start(out=nc.lookup_mls('dbg3').memorylocations[0].bass_handle.ap() if False else dbg3.ap(), in_=payS[:])
                        nc.sync.dma_start(out=dbg4.ap(), in_=msg[:])
                    nc.tensor.matmul(accp[:, 0:P], lhsT=payS[:, 0:P],
                                     rhs=en[:, c * P:(c + 1) * P],
                                     start=(c == 0), stop=(c == nch - 1))
                    nc.tensor.matmul(accp[0:64, P:2 * P],
                                     lhsT=payS[:, P:HID],
                                     rhs=en[:, c * P:(c + 1) * P],
                                     start=False, stop=(c == nch - 1))
                    nc.tensor.matmul(accp[0:4, 2 * P:3 * P],
                                     lhsT=payS[:, HID:196],
                                     rhs=en[:, c * P:(c + 1) * P],
                                     start=False, stop=(c == nch - 1))
                nc.vector.tensor_tensor(out=hacc[:, 0:P], in0=hacc[:, 0:P],
                                        in1=accp[:, 0:P], op=OP.add)
                nc.vector.tensor_tensor(out=hacc[0:64, P:2 * P],
                                        in0=hacc[0:64, P:2 * P],
                                        in1=accp[0:64, P:2 * P], op=OP.add)
                nc.vector.tensor_tensor(out=hacc[0:4, 2 * P:3 * P],
                                        in0=hacc[0:4, 2 * P:3 * P],
                                        in1=accp[0:4, 2 * P:3 * P], op=OP.add)

            if li < 2:
                nc.gpsimd.collective_compute(
                    'AllGather', OP.bypass,
                    replica_groups=[list(range(NCORES))],
                    ins=[shard_out[li][:]],
                    outs=[Tnext[li][:]])

        nc.sync.dma_start(out=pooled.ap(), in_=pool_acc[:])
        if dbg:
            nc.sync.dma_start(out=dbg0.ap(), in_=shard_out[0][:])
            nc.sync.dma_start(out=dbg1.ap(), in_=shard_out[1][:])

    nc.compile()
    return nc


def _node_stage(nc, sb, ps, W, li, t, shard, hacc, ident, sel3, iotaG,
                batch_t, pool_acc, shard_out):
    n1k0, n1k1, n1k2 = W[f'nw1k0_{li}'], W[f'nw1k1_{li}'], W[f'nw1k2_{li}']
    n2k0, n2k1 = W[f'nw2k0_{li}'], W[f'nw2k1_{li}']
    shTp = ps.tile([D, P], F32, tag='ps', bufs=3, padded_shape=[P, 512])
    nc.tensor.transpose(out=shTp[:], in_=shard[:], identity=ident[:])
    nfeat = sb.tile([97, P], F32, tag='nfeat')
    nc.vector.tensor_copy(nfeat[0:FPAD, :], shTp[0:FPAD, :])
    nc.gpsimd.memset(nfeat[FPAD:97, :], 1.0)
    tp = ps.tile([FPAD, 256], F32, tag='psT', bufs=2, padded_shape=[P, 512])
    for m in range(2):
        ms = slice(m * FPAD, (m + 1) * FPAD)
        nc.tensor.matmul(tp[:, m * P:(m + 1) * P], lhsT=n1k0[:, ms],
                         rhs=nfeat[:], start=True, stop=False)
        nc.tensor.matmul(tp[:, m * P:(m + 1) * P], lhsT=n1k1[:, ms],
                         rhs=hacc[:, 0:P], start=False, stop=False)
        nc.tensor.matmul(tp[:, m * P:(m + 1) * P], lhsT=n1k2[:, ms],
                         rhs=hacc[0:64, P:2 * P], start=False, stop=True)
    tS = sb.tile([97, 256], F32, tag='tS')
    nc.scalar.activation(tS[0:FPAD, :], tp[:], AF.Silu)
    nc.gpsimd.memset(tS[FPAD:97, 0:P], 1.0)
    h2p = ps.tile([FPAD, P], F32, tag='ps', bufs=3, padded_shape=[P, 512])
    nc.tensor.matmul(h2p[:], lhsT=n2k0[:], rhs=tS[0:97, 0:P],
                     start=True, stop=False)
    nc.tensor.matmul(h2p[:], lhsT=n2k1[:], rhs=tS[0:FPAD, P:256],
                     start=False, stop=True)
    combT = sb.tile([D, P], F32, tag='combT')
    nc.scalar.activation(combT[0:FPAD, :], h2p[:], AF.Relu)
    if li == 0 and t == 0 and hasattr(nc, '_dbg2'):
        nc.sync.dma_start(out=nc._dbg2.ap(), in_=hacc[:])
    dmx = sb.tile([4, P], F32, tag='dmx')
    nc.vector.tensor_scalar_max(out=dmx[:], in0=hacc[0:4, 2 * P:3 * P],
                                scalar1=1.0)
    rec = sb.tile([4, P], F32, tag='rec')
    nc.vector.reciprocal_approx_fast(out=rec[:], in_=dmx[:])
    rb3 = ps.tile([POS, P], F32, tag='psT', bufs=2, padded_shape=[P, 512])
    nc.tensor.matmul(rb3[:], lhsT=sel3[:], rhs=rec[:], start=True,
                     stop=True)
    xm = sb.tile([POS, P], F32, tag='xm')
    nc.vector.tensor_tensor(out=xm[:], in0=hacc[0:POS, 2 * P:3 * P],
                            in1=rb3[:], op=OP.mult)
    nc.vector.memset(combT[FPAD:D, :], 0.0)
    nc.vector.tensor_tensor(out=combT[FPAD:FPAD + POS, :], in0=xm[:],
                            in1=shTp[FPAD:FPAD + POS, :], op=OP.add)
    cmTp = ps.tile([P, D], F32, tag='ps', bufs=3, padded_shape=[P, 512])
    nc.tensor.transpose(out=cmTp[:], in_=combT[:],
                        identity=ident[0:D, 0:D])
    comb = sb.tile([P, D], F32, tag='comb')
    nc.scalar.copy(comb[:], cmTp[:])
    if li < 2:
        nc.sync.dma_start(out=shard_out[li][t * P:(t + 1) * P, :],
                          in_=comb[:])
    else:
        og = sb.tile([P, G], F32, tag='og')
        nc.vector.tensor_tensor(
            out=og[:], in0=batch_t[:, t:t + 1].to_broadcast([P, G]),
            in1=iotaG[:], op=OP.is_equal)
        pp = ps.tile([G, OUT + POS], F32, tag='psT', bufs=2, padded_shape=[P, 512])
        nc.tensor.matmul(pp[:], lhsT=og[:], rhs=comb[:, 0:OUT + POS],
                         start=True, stop=True)
        nc.vector.tensor_tensor(out=pool_acc[:], in0=pool_acc[:], in1=pp[:],
                                op=OP.add)


# ------------------------------------------------------------------- driver
_CACHE = {}


def get_compiled(plan):
    key = (plan['N'], plan['F0'], plan['CH'], plan['blocks'])
    if key not in _CACHE:
        _CACHE[key] = _build(plan)
    return _CACHE[key]


def prepare(feat, coord, edge_attr, edge_index, batch, params):
    plan, shared, core_inputs, head = _prep(feat, coord, edge_attr,
                                            edge_index, batch, params)
    nc = get_compiled(plan)
    in_maps = []
    for c in range(NCORES):
        m = dict(shared)
        m.update(core_inputs[c])
        in_maps.append(m)
    return nc, in_maps, head


def finish(results, head):
    pooled = np.zeros((G, OUT + POS), np.float32)
    for r in results:
        pooled += r['pooled']
    hw1, hb1, hw2, hb2 = head
    h = np.maximum(pooled @ hw1 + hb1, 0.0)
    return (h @ hw2 + hb2).astype(np.float32)


def kernel(feat, coord, edge_attr, edge_index, batch, params):
    nc, in_maps, head = prepare(feat, coord, edge_attr, edge_index, batch,
                                params)
    res = run_bass_kernel_spmd(nc, in_maps, core_ids=list(range(NCORES)))
    return finish(res.results, head)


# revision 23
# speedup vs baseline: 1.0076x; 1.0076x over previous
"""EGNN (3-layer EGNNConv + global_add_pool + MLP head) on 8 TRN2 NeuronCores.

Sharding: nodes are split into 8 contiguous shards (dst-owner sharding);
edges are sorted by dst on the host, assigned to the owning core, and
grouped by 128-node destination tiles. Per 512-edge block, feat/coord[src]
come from an indirect-DMA gather out of a replicated DRAM node table,
feat/coord[dst] from a one-hot matmul against the core-local shard tile,
and the segment-sum scatter is a one-hot matmul accumulated in PSUM.
Updated node features are all-gathered between layers. The device emits
per-core global_add_pool partials [64, 99]; the tiny head MLP runs on host.
"""
from contextlib import ExitStack

import numpy as np

import concourse.bass as bass
from concourse import bacc, mybir
from concourse.bass_utils import run_bass_kernel_spmd
from concourse.masks import make_identity
from concourse.tile import TileContext

F32 = mybir.dt.float32
I32 = mybir.dt.int32
AF = mybir.ActivationFunctionType
OP = mybir.AluOpType

NCORES = 8
P = 128
D = 100          # node-table row: feat 0:96, coord 96:99, pad 99
FPAD = 96        # padded node-feature width (= OUT)
HID = 192
OUT = 96
POS = 3
EF = 16
G = 64
KE1 = 2 * FPAD + 1 + EF   # 209 edge-MLP input rows


def _npf(x):
    return np.ascontiguousarray(np.asarray(x), dtype=np.float32)


# ----------------------------------------------------------------- host prep
def _prep(feat, coord, edge_attr, edge_index, batch, params):
    N = feat.shape[0]
    F0 = feat.shape[1]
    NSH = -(-N // NCORES)
    NT = -(-NSH // P)
    NSHP = NT * P
    NTAB = NCORES * NSHP

    src = np.asarray(edge_index[0]).astype(np.int64)
    dst = np.asarray(edge_index[1]).astype(np.int64)
    ea = _npf(edge_attr)
    order = np.argsort(dst, kind='stable')
    src_s, dst_s, ea_s = src[order], dst[order], ea[order]

    cb = np.searchsorted(dst_s, np.arange(NCORES + 1) * NSH)
    counts = np.zeros((NCORES, NT), np.int64)
    percore = []
    for c in range(NCORES):
        sl = slice(cb[c], cb[c + 1])
        drel = dst_s[sl] - c * NSH
        tl = drel // P
        counts[c] = np.bincount(tl, minlength=NT)
        percore.append((src_s[sl], drel, ea_s[sl], tl))

    chunks_t = np.maximum(1, -(-counts.max(0) // P)).astype(np.int64)
    chunk_base = np.concatenate([[0], np.cumsum(chunks_t)])
    CH = int(chunk_base[-1])
    EPAD = CH * P

    blocks = []   # (tile, nch, ch0)
    for t in range(NT):
        rem, ch0 = int(chunks_t[t]), int(chunk_base[t])
        while rem > 0:
            nch = min(4, rem)
            blocks.append((t, nch, ch0))
            ch0 += nch
            rem -= nch

    batch_np = np.asarray(batch).astype(np.int64)

    T0 = np.zeros((NTAB, D), np.float32)
    fe = _npf(feat)
    co = _npf(coord)
    for c in range(NCORES):
        n0, n1 = c * NSH, min((c + 1) * NSH, N)
        r0 = c * NSHP
        T0[r0:r0 + (n1 - n0), 0:F0] = fe[n0:n1]
        T0[r0:r0 + (n1 - n0), FPAD:FPAD + POS] = co[n0:n1]

    core_inputs = []
    for c in range(NCORES):
        ss, drel, eas, tl = percore[c]
        tf = np.searchsorted(tl, np.arange(NT))
        pos = np.arange(len(tl))
        rank = pos - tf[tl]
        dsti = chunk_base[tl] * P + rank
        srcg = np.zeros(EPAD, np.int32)
        srcg[dsti] = ((ss // NSH) * NSHP + (ss % NSH)).astype(np.int32)
        drelf = np.full(EPAD, -1.0, np.float32)
        drelf[dsti] = (drel - tl * P).astype(np.float32)
        eaT = np.zeros((EF + 1, EPAD), np.float32)
        eaT[0:EF, dsti] = eas.T
        eaT[EF, :] = 1.0
        slots = np.arange(NSHP)
        node = np.minimum(c * NSH + slots, N - 1)
        br = np.where((slots < NSH) & (c * NSH + slots < N),
                      batch_np[node], 999).astype(np.float32)
        core_inputs.append({
            'srcI': np.ascontiguousarray(srcg.reshape(CH, P).T),
            'dstR': np.ascontiguousarray(drelf.reshape(CH, P).T),
            'eaT': eaT,
            'batchR': np.ascontiguousarray(br.reshape(NT, P).T),
            'own_tab': np.ascontiguousarray(T0[c * NSHP:(c + 1) * NSHP]),
        })

    layers = []
    in_sizes = [F0, OUT, OUT]
    for li, lp in enumerate(params['layers']):
        fi = in_sizes[li]
        w1, b1 = map(_npf, lp['ew1'])
        w2, b2 = map(_npf, lp['ew2'])
        cc1, c1b = map(_npf, lp['cw1'])
        cc2 = _npf(lp['cw2'][0])
        nn1, n1b = map(_npf, lp['nw1'])
        nn2, n2b = map(_npf, lp['nw2'])
        W1p = np.zeros((KE1, HID), np.float32)
        W1p[0:fi] = w1[0:fi]
        W1p[FPAD:FPAD + fi] = w1[fi:2 * fi]
        W1p[2 * FPAD] = w1[2 * fi]
        W1p[2 * FPAD + 1:KE1] = w1[2 * fi + 1:]
        NW1p = np.zeros((FPAD + HID, HID), np.float32)
        NW1p[0:fi] = nn1[0:fi]
        NW1p[FPAD:FPAD + HID] = nn1[fi:fi + HID]
        layers.append({
            f'ew1k0_{li}': W1p[0:P],
            f'ew1k1_{li}': np.vstack([W1p[P:KE1], b1[None, :]]),      # [82,192]
            f'ew2k0_{li}': np.vstack([w2[0:FPAD], b2[None, :]]),      # [97,192]
            f'ew2k1_{li}': w2[FPAD:HID],                              # [96,192]
            f'cw1k0_{li}': np.vstack([cc1[0:FPAD], c1b[None, :]]),
            f'cw1k1_{li}': cc1[FPAD:HID],
            f'cw2k0_{li}': cc2[0:FPAD],                               # [96,1]
            f'cw2k1_{li}': cc2[FPAD:HID],
            f'nw1k0_{li}': np.vstack([NW1p[0:FPAD], n1b[None, :]]),   # [97,192]
            f'nw1k1_{li}': NW1p[FPAD:FPAD + P],                       # [128,192]
            f'nw1k2_{li}': NW1p[FPAD + P:FPAD + HID],                 # [64,192]
            f'nw2k0_{li}': np.vstack([nn2[0:FPAD], n2b[None, :]]),    # [97,96]
            f'nw2k1_{li}': nn2[FPAD:HID],                             # [96,96]
        })
    wdict = {}
    for lw in layers:
        wdict.update({k: np.ascontiguousarray(v) for k, v in lw.items()})

    head_w1, head_b1 = map(_npf, params['head']['w1'])
    head_w2, head_b2 = map(_npf, params['head']['w2'])

    plan = {
        'N': N, 'F0': F0, 'NSH': NSH, 'NT': NT, 'NSHP': NSHP,
        'NTAB': NTAB, 'CH': CH, 'EPAD': EPAD, 'blocks': tuple(blocks),
    }
    shared = dict(wdict)
    shared['T0'] = T0
    head = (head_w1, head_b1, head_w2, head_b2)
    return plan, shared, core_inputs, head


# -------------------------------------------------------------- bass builder
WSHAPES = [('ew1k0', [P, HID]), ('ew1k1', [82, HID]),
           ('ew2k0', [97, HID]), ('ew2k1', [96, HID]),
           ('cw1k0', [97, HID]), ('cw1k1', [96, HID]),
           ('cw2k0', [96, 1]), ('cw2k1', [96, 1]),
           ('nw1k0', [97, HID]), ('nw1k1', [P, HID]),
           ('nw1k2', [64, HID]), ('nw2k0', [97, OUT]),
           ('nw2k1', [96, OUT])]


def _fast_rsqrt(nc, pool, r_ap, nch):
    """y = 1/sqrt(max(r,1e-30)); token-major [128, nch]; all on DVE."""
    rcl = pool.tile([P, 4], F32, tag='rs_rcl')
    nc.vector.tensor_scalar_max(out=rcl[:, 0:nch], in0=r_ap, scalar1=1e-30)
    yi = pool.tile([P, 4], I32, tag='rs_yi')
    nc.vector.tensor_scalar(out=yi[:, 0:nch], in0=rcl[:, 0:nch].bitcast(I32),
                            scalar1=1, scalar2=None, op0=OP.arith_shift_right)
    nc.vector.tensor_scalar(out=yi[:, 0:nch], in0=yi[:, 0:nch], scalar1=-1,
                            scalar2=0x5f3759df, op0=OP.mult, op1=OP.add)
    y = yi[:, 0:nch].bitcast(F32)
    h = pool.tile([P, 4], F32, tag='rs_h')
    nc.vector.tensor_scalar(out=h[:, 0:nch], in0=rcl[:, 0:nch], scalar1=0.5,
                            scalar2=None, op0=OP.mult)
    t = pool.tile([P, 4], F32, tag='rs_t')
    for _ in range(2):
        nc.vector.tensor_tensor(out=t[:, 0:nch], in0=y, in1=y, op=OP.mult)
        nc.vector.tensor_tensor(out=t[:, 0:nch], in0=t[:, 0:nch],
                                in1=h[:, 0:nch], op=OP.mult)
        nc.vector.tensor_scalar(out=t[:, 0:nch], in0=t[:, 0:nch],
                                scalar1=-1.0, scalar2=1.5, op0=OP.mult,
                                op1=OP.add)
        nc.vector.tensor_tensor(out=y, in0=y, in1=t[:, 0:nch], op=OP.mult)
    return yi


def _build(plan, n_devices=NCORES, with_collectives=True, stage=3):
    NT, NSHP, NTAB = plan['NT'], plan['NSHP'], plan['NTAB']
    CH, EPAD = plan['CH'], plan['EPAD']
    blocks = list(plan['blocks'])

    nc = bacc.Bacc('TRN2', target_bir_lowering=False, debug=False,
                   num_devices=n_devices)

    T0 = nc.dram_tensor('T0', [NTAB, D], F32, kind='ExternalInput')
    own_tab = nc.dram_tensor('own_tab', [NSHP, D], F32, kind='ExternalInput')
    srcI = nc.dram_tensor('srcI', [P, CH], I32, kind='ExternalInput')
    dstR = nc.dram_tensor('dstR', [P, CH], F32, kind='ExternalInput')
    eaT = nc.dram_tensor('eaT', [EF + 1, EPAD], F32,
                     kind='ExternalInput')
    batchR = nc.dram_tensor('batchR', [P, NT], F32, kind='ExternalInput')
    wt = {}
    for li in range(3):
        for nm, shp in WSHAPES:
            wt[f'{nm}_{li}'] = nc.dram_tensor(f'{nm}_{li}', shp, F32,
                                              kind='ExternalInput')
    pooled = nc.dram_tensor('pooled', [G, OUT + POS], F32,
                            kind='ExternalOutput')
    import os
    dbg = os.environ.get('KERNEL_DBG') == '1'
    if dbg:
        dbg0 = nc.dram_tensor('dbg0', [NSHP, D], F32, kind='ExternalOutput')
        dbg1 = nc.dram_tensor('dbg1', [NSHP, D], F32, kind='ExternalOutput')
        dbg2 = nc.dram_tensor('dbg2', [P, 384], F32, kind='ExternalOutput')
        nc._dbg2 = dbg2
        dbg3 = nc.dram_tensor('dbg3', [P, 196], F32, kind='ExternalOutput')
        dbg4 = nc.dram_tensor('dbg4', [97, 1024], F32, kind='ExternalOutput')

    with TileContext(nc) as tc, ExitStack() as ctx:
        cp = ctx.enter_context(tc.tile_pool(name='const', bufs=1))
        sb = ctx.enter_context(tc.tile_pool(name='work', bufs=3))
        sb3 = ctx.enter_context(tc.tile_pool(name='work3', bufs=3))
        ps = ctx.enter_context(tc.tile_pool(name='psA', bufs=3, space='PSUM'))
        pa = ctx.enter_context(tc.tile_pool(name='psacc', bufs=1,
                                            space='PSUM'))
        pb = ctx.enter_context(tc.tile_pool(name='psB', bufs=2, space='PSUM'))
        dr = ctx.enter_context(tc.tile_pool(name='dram', bufs=1,
                                            space='DRAM'))

        ident = cp.tile([P, P], F32)
        make_identity(nc, ident[:])
        iota_row = cp.tile([P, P], F32)
        nc.gpsimd.iota(iota_row[:], pattern=[[1, P]], base=0,
                       channel_multiplier=0,
                       allow_small_or_imprecise_dtypes=True)
        iotaG = cp.tile([P, G], F32)
        nc.gpsimd.iota(iotaG[:], pattern=[[1, G]], base=0,
                       channel_multiplier=0,
                       allow_small_or_imprecise_dtypes=True)
        ones1x3 = cp.tile([1, POS], F32)
        nc.vector.memset(ones1x3[:], 1.0)
        sel3 = cp.tile([4, POS], F32)
        nc.gpsimd.iota(sel3[:], pattern=[[0, POS]], base=-3,
                       channel_multiplier=1,
                       allow_small_or_imprecise_dtypes=True)
        nc.vector.tensor_scalar(out=sel3[:], in0=sel3[:], scalar1=0.0,
                                scalar2=None, op0=OP.is_equal)

        src_t = cp.tile([P, CH], I32)
        nc.sync.dma_start(out=src_t[:], in_=srcI.ap())
        dst_t = cp.tile([P, CH], F32)
        nc.sync.dma_start(out=dst_t[:], in_=dstR.ap())
        batch_t = cp.tile([P, NT], F32)
        nc.sync.dma_start(out=batch_t[:], in_=batchR.ap())

        W = {}
        for li in range(3):
            for nm, shp in WSHAPES:
                key = f'{nm}_{li}'
                w = cp.tile(shp, F32, name=f'w_{key}')
                nc.sync.dma_start(out=w[:], in_=wt[key].ap())
                W[key] = w

        shard_out = [dr.tile([NSHP, D], F32, name=f'shard_out_{li}')
                     for li in range(2)]
        Tnext = [dr.tile([NTAB, D], F32, addr_space='Shared',
                         name=f'T_{li + 1}') for li in range(2)]

        pool_acc = cp.tile([G, OUT + POS], F32)
        nc.vector.memset(pool_acc[:], 0.0)

        for li in range(3):
            Tcur = T0.ap() if li == 0 else Tnext[li - 1][:]
            own_cur = own_tab.ap() if li == 0 else shard_out[li - 1][:]
            e1k0, e1k1 = W[f'ew1k0_{li}'], W[f'ew1k1_{li}']
            e2k0, e2k1 = W[f'ew2k0_{li}'], W[f'ew2k1_{li}']
            c1k0, c1k1 = W[f'cw1k0_{li}'], W[f'cw1k1_{li}']
            c2k0, c2k1 = W[f'cw2k0_{li}'], W[f'cw2k1_{li}']

            cur_tile = -1
            shard = hacc = None
            for (t, nch, ch0) in blocks + [(-1, 0, 0)]:
                if t != cur_tile:
                    if cur_tile >= 0:
                        _node_stage(nc, sb, ps, W, li, cur_tile, shard, hacc,
                                    ident, sel3, iotaG, batch_t, pool_acc,
                                    shard_out)
                    if t < 0:
                        break
                    cur_tile = t
                    shard = sb3.tile([P, D], F32, tag='shard')
                    nc.sync.dma_start(out=shard[:],
                                      in_=own_cur[t * P:(t + 1) * P, :])
                    # hacc: [:, 0:128] = feats 0:128 x nodes;
                    #       [0:68, 128:256] = feats 128:196 x nodes
                    hacc = sb3.tile([P, 384], F32, tag='hacc')
                    nc.gpsimd.memset(hacc[:], 0.0)

                NE = nch * P
                eoff = ch0 * P
                g = sb.tile([P, 4 * D], F32, tag='g', bufs=8)
                for c in range(nch):
                    nc.gpsimd.indirect_dma_start(
                        out=g[:, c * D:(c + 1) * D], out_offset=None,
                        in_=Tcur,
                        in_offset=bass.IndirectOffsetOnAxis(
                            ap=src_t[:, ch0 + c:ch0 + c + 1], axis=0))
                if stage < 1:
                    continue
                srcTp = ps.tile([D, 512], F32, tag='ps', bufs=3, padded_shape=[P, 512])
                for c in range(nch):
                    nc.tensor.transpose(out=srcTp[:, c * P:(c + 1) * P],
                                        in_=g[:, c * D:(c + 1) * D],
                                        identity=ident[:])
                en = sb.tile([P, 512], F32, tag='en', bufs=3)
                nc.vector.tensor_tensor(
                    out=en[:, 0:NE].rearrange('p (c e) -> p c e', c=nch),
                    in0=dst_t[:, ch0:ch0 + nch].unsqueeze(2)
                        .to_broadcast([P, nch, P]),
                    in1=iota_row[:].unsqueeze(1).to_broadcast([P, nch, P]),
                    op=OP.is_equal)
                nep = ps.tile([P, 512], F32, tag='ps', bufs=3, padded_shape=[P, 512])
                for c in range(nch):
                    nc.tensor.transpose(out=nep[:, c * P:(c + 1) * P],
                                        in_=en[:, c * P:(c + 1) * P],
                                        identity=ident[:])
                ne = sb.tile([P, 512], F32, tag='ne')
                nc.scalar.copy(ne[:, 0:NE], nep[:, 0:NE])
                dstF = ps.tile([D, 512], F32, tag='ps', bufs=3, padded_shape=[P, 512])
                nc.tensor.matmul(dstF[:, 0:NE], lhsT=shard[:],
                                 rhs=ne[:, 0:NE], start=True, stop=True)
                K0 = sb.tile([P, 512], F32, tag='K0', bufs=3)
                nc.scalar.copy(K0[0:FPAD, 0:NE], srcTp[0:FPAD, 0:NE])
                nc.vector.tensor_copy(K0[FPAD:P, 0:NE], dstF[0:32, 0:NE])
                K1 = sb.tile([82, 512], F32, tag='K1', bufs=3)
                nc.vector.tensor_copy(K1[0:32, 0:NE], dstF[32:64, 0:NE])
                nc.vector.tensor_copy(K1[32:64, 0:NE], dstF[64:FPAD, 0:NE])
                nc.sync.dma_start(out=K1[65:82, 0:NE],
                                  in_=eaT.ap()[:, eoff:eoff + NE])
                cD = sb.tile([POS, 512], F32, tag='cD')
                nc.scalar.copy(cD[:, 0:NE], dstF[FPAD:FPAD + POS, 0:NE])
                xd = sb.tile([POS, 512], F32, tag='xd')
                nc.vector.tensor_tensor(out=xd[:, 0:NE],
                                        in0=srcTp[FPAD:FPAD + POS, 0:NE],
                                        in1=cD[:, 0:NE], op=OP.subtract)
                # radial & rsqrt, token-major
                xdTp = ps.tile([P, 12], F32, tag='psT', bufs=2, padded_shape=[P, 512])
                for c in range(nch):
                    nc.tensor.transpose(out=xdTp[:, c * POS:(c + 1) * POS],
                                        in_=xd[:, c * P:(c + 1) * P],
                                        identity=ident[0:POS, 0:POS])
                xdtm = sb.tile([P, 12], F32, tag='xdtm')
                nc.vector.tensor_copy(xdtm[:, 0:nch * POS],
                                      xdTp[:, 0:nch * POS])
                sq = sb.tile([P, 12], F32, tag='sq')
                nc.vector.tensor_tensor(out=sq[:, 0:nch * POS],
                                        in0=xdtm[:, 0:nch * POS],
                                        in1=xdtm[:, 0:nch * POS], op=OP.mult)
                rtm = sb.tile([P, 4], F32, tag='rtm')
                nc.vector.reduce_sum(
                    rtm[:, 0:nch],
                    sq[:, 0:nch * POS].rearrange('p (c k) -> p c k', k=POS),
                    axis=mybir.AxisListType.X)
                radp = ps.tile([1, 512], F32, tag='psT', bufs=2, padded_shape=[P, 512])
                for c in range(nch):
                    nc.tensor.transpose(out=radp[:, c * P:(c + 1) * P],
                                        in_=rtm[:, c:c + 1],
                                        identity=ident[:])
                nc.vector.tensor_copy(K1[64:65, 0:NE], radp[:, 0:NE])
                ytile = _fast_rsqrt(nc, sb, rtm[:, 0:nch], nch)
                rnp = ps.tile([1, 512], F32, tag='psT', bufs=2, padded_shape=[P, 512])
                for c in range(nch):
                    nc.tensor.transpose(out=rnp[:, c * P:(c + 1) * P],
                                        in_=ytile[0:P, c:c + 1].bitcast(F32),
                                        identity=ident[:])
                rn = sb.tile([1, 512], F32, tag='rn')
                nc.vector.tensor_copy(rn[:, 0:NE], rnp[:, 0:NE])
                # edge MLP (fixed 0/512 offsets keep PSUM writes bank-aligned)
                if stage < 2:
                    continue
                h1 = sb.tile([97, 1024], F32, tag='h1')
                for m in range(2):
                    ms = slice(m * FPAD, (m + 1) * FPAD)
                    o = m * 512
                    h1p = pb.tile([FPAD, 512], F32, tag='pb', bufs=2,
                                  padded_shape=[P, 512])
                    nc.tensor.matmul(h1p[:, 0:NE], lhsT=e1k0[:, ms],
                                     rhs=K0[:, 0:NE], start=True, stop=False)
                    nc.tensor.matmul(h1p[:, 0:NE], lhsT=e1k1[:, ms],
                                     rhs=K1[:, 0:NE], start=False, stop=True)
                    nc.scalar.activation(h1[0:FPAD, o:o + NE], h1p[:, 0:NE],
                                         AF.Silu)
                nc.gpsimd.memset(h1[FPAD:97, 0:NE], 1.0)
                msg = sb.tile([97, 1024], F32, tag='msg')
                for m in range(2):
                    ms = slice(m * FPAD, (m + 1) * FPAD)
                    o = m * 512
                    msgp = pb.tile([FPAD, 512], F32, tag='pb', bufs=2,
                                   padded_shape=[P, 512])
                    nc.tensor.matmul(msgp[:, 0:NE], lhsT=e2k0[:, ms],
                                     rhs=h1[0:97, 0:NE], start=True,
                                     stop=False)
                    nc.tensor.matmul(msgp[:, 0:NE], lhsT=e2k1[:, ms],
                                     rhs=h1[0:FPAD, 512:512 + NE],
                                     start=False, stop=True)
                    nc.scalar.activation(msg[0:FPAD, o:o + NE], msgp[:, 0:NE],
                                         AF.Silu)
                nc.gpsimd.memset(msg[FPAD:97, 0:NE], 1.0)
                c1t = sb.tile([FPAD, 1024], F32, tag='c1t')
                for m in range(2):
                    ms = slice(m * FPAD, (m + 1) * FPAD)
                    o = m * 512
                    c1p = pb.tile([FPAD, 512], F32, tag='pb', bufs=2,
                                  padded_shape=[P, 512])
                    nc.tensor.matmul(c1p[:, 0:NE], lhsT=c1k0[:, ms],
                                     rhs=msg[0:97, 0:NE], start=True,
                                     stop=False)
                    nc.tensor.matmul(c1p[:, 0:NE], lhsT=c1k1[:, ms],
                                     rhs=msg[0:FPAD, 512:512 + NE],
                                     start=False, stop=True)
                    nc.scalar.activation(c1t[:, o:o + NE], c1p[:, 0:NE],
                                         AF.Silu)
                wp = ps.tile([1, 512], F32, tag='psT', bufs=2, padded_shape=[P, 512])
                nc.tensor.matmul(wp[:, 0:NE], lhsT=c2k0[:], rhs=c1t[:, 0:NE],
                                 start=True, stop=False)
                nc.tensor.matmul(wp[:, 0:NE], lhsT=c2k1[:],
                                 rhs=c1t[:, 512:512 + NE], start=False,
                                 stop=True)
                wrn = sb.tile([1, 512], F32, tag='wrn')
                nc.vector.tensor_tensor(out=wrn[:, 0:NE], in0=rn[:, 0:NE],
                                        in1=wp[:, 0:NE], op=OP.mult)
                wb3 = ps.tile([POS, 512], F32, tag='psT', bufs=2, padded_shape=[P, 512])
                nc.tensor.matmul(wb3[:, 0:NE], lhsT=ones1x3[:],
                                 rhs=wrn[:, 0:NE], start=True, stop=True)
                extras = sb.tile([4, 512], F32, tag='extras')
                nc.vector.tensor_tensor(out=extras[0:POS, 0:NE],
                                        in0=xd[:, 0:NE], in1=wb3[:, 0:NE],
                                        op=OP.mult)
                nc.sync.dma_start(out=extras[POS:4, 0:NE],
                                  in_=eaT.ap()[EF:EF + 1, eoff:eoff + NE])
                if stage < 3:
                    continue
                accp = pa.tile([P, 384], F32, tag='acc', bufs=1, padded_shape=[P, 512])
                for c in range(nch):
                    payT = ps.tile([P, 196], F32, tag='ps', bufs=3, padded_shape=[P, 512])
                    nc.tensor.transpose(
                        out=payT[:, 0:FPAD],
                        in_=msg[0:FPAD, c * P:(c + 1) * P],
                        identity=ident[0:FPAD, 0:FPAD])
                    nc.tensor.transpose(
                        out=payT[:, FPAD:2 * FPAD],
                        in_=msg[0:FPAD, 512 + c * P:512 + (c + 1) * P],
                        identity=ident[0:FPAD, 0:FPAD])
                    nc.tensor.transpose(
                        out=payT[:, 2 * FPAD:196],
                        in_=extras[:, c * P:(c + 1) * P],
                        identity=ident[0:4, 0:4])
                    payS = sb.tile([P, 196], F32, tag='payS', bufs=4)
                    nc.vector.tensor_copy(payS[:], payT[:])
                    if dbg and li == 0 and ch0 == 0 and c == 0:
                        nc.sync.dma_start(out=nc.lookup_mls('dbg3').memorylocations[0].bass_handle.ap() if False else dbg3.ap(), in_=payS[:])
                        nc.sync.dma_start(out=dbg4.ap(), in_=msg[:])
                    nc.tensor.matmul(accp[:, 0:P], lhsT=payS[:, 0:P],
                                     rhs=en[:, c * P:(c + 1) * P],
                                     start=(c == 0), stop=(c == nch - 1))
                    nc.tensor.matmul(accp[0:64, P:2 * P],
                                     lhsT=payS[:, P:HID],
                                     rhs=en[:, c * P:(c + 1) * P],
                                     start=False, stop=(c == nch - 1))
                    nc.tensor.matmul(accp[0:4, 2 * P:3 * P],
                                     lhsT=payS[:, HID:196],
                                     rhs=en[:, c * P:(c + 1) * P],
                                     start=False, stop=(c == nch - 1))
                nc.vector.tensor_tensor(out=hacc[:, 0:P], in0=hacc[:, 0:P],
                                        in1=accp[:, 0:P], op=OP.add)
                nc.vector.tensor_tensor(out=hacc[0:64, P:2 * P],
                                        in0=hacc[0:64, P:2 * P],
                                        in1=accp[0:64, P:2 * P], op=OP.add)
                nc.vector.tensor_tensor(out=hacc[0:4, 2 * P:3 * P],
                                        in0=hacc[0:4, 2 * P:3 * P],
                                        in1=accp[0:4, 2 * P:3 * P], op=OP.add)

            if li < 2 and with_collectives:
                nc.gpsimd.collective_compute(
                    'AllGather', OP.bypass,
                    replica_groups=[list(range(NCORES))],
                    ins=[shard_out[li][:]],
                    outs=[Tnext[li][:]])

        nc.sync.dma_start(out=pooled.ap(), in_=pool_acc[:])
        if dbg:
            nc.sync.dma_start(out=dbg0.ap(), in_=shard_out[0][:])
            nc.sync.dma_start(out=dbg1.ap(), in_=shard_out[1][:])

    nc.compile()
    return nc


def _node_stage(nc, sb, ps, W, li, t, shard, hacc, ident, sel3, iotaG,
                batch_t, pool_acc, shard_out):
    n1k0, n1k1, n1k2 = W[f'nw1k0_{li}'], W[f'nw1k1_{li}'], W[f'nw1k2_{li}']
    n2k0, n2k1 = W[f'nw2k0_{li}'], W[f'nw2k1_{li}']
    shTp = ps.tile([D, P], F32, tag='ps', bufs=3, padded_shape=[P, 512])
    nc.tensor.transpose(out=shTp[:], in_=shard[:], identity=ident[:])
    nfeat = sb.tile([97, P], F32, tag='nfeat')
    nc.vector.tensor_copy(nfeat[0:FPAD, :], shTp[0:FPAD, :])
    nc.gpsimd.memset(nfeat[FPAD:97, :], 1.0)
    tp = ps.tile([FPAD, 256], F32, tag='psT', bufs=2, padded_shape=[P, 512])
    for m in range(2):
        ms = slice(m * FPAD, (m + 1) * FPAD)
        nc.tensor.matmul(tp[:, m * P:(m + 1) * P], lhsT=n1k0[:, ms],
                         rhs=nfeat[:], start=True, stop=False)
        nc.tensor.matmul(tp[:, m * P:(m + 1) * P], lhsT=n1k1[:, ms],
                         rhs=hacc[:, 0:P], start=False, stop=False)
        nc.tensor.matmul(tp[:, m * P:(m + 1) * P], lhsT=n1k2[:, ms],
                         rhs=hacc[0:64, P:2 * P], start=False, stop=True)
    tS = sb.tile([97, 256], F32, tag='tS')
    nc.scalar.activation(tS[0:FPAD, :], tp[:], AF.Silu)
    nc.gpsimd.memset(tS[FPAD:97, 0:P], 1.0)
    h2p = ps.tile([FPAD, P], F32, tag='ps', bufs=3, padded_shape=[P, 512])
    nc.tensor.matmul(h2p[:], lhsT=n2k0[:], rhs=tS[0:97, 0:P],
                     start=True, stop=False)
    nc.tensor.matmul(h2p[:], lhsT=n2k1[:], rhs=tS[0:FPAD, P:256],
                     start=False, stop=True)
    combT = sb.tile([D, P], F32, tag='combT')
    nc.scalar.activation(combT[0:FPAD, :], h2p[:], AF.Relu)
    if li == 0 and t == 0 and hasattr(nc, '_dbg2'):
        nc.sync.dma_start(out=nc._dbg2.ap(), in_=hacc[:])
    dmx = sb.tile([4, P], F32, tag='dmx')
    nc.vector.tensor_scalar_max(out=dmx[:], in0=hacc[0:4, 2 * P:3 * P],
                                scalar1=1.0)
    rec = sb.tile([4, P], F32, tag='rec')
    nc.vector.reciprocal_approx_fast(out=rec[:], in_=dmx[:])
    rb3 = ps.tile([POS, P], F32, tag='psT', bufs=2, padded_shape=[P, 512])
    nc.tensor.matmul(rb3[:], lhsT=sel3[:], rhs=rec[:], start=True,
                     stop=True)
    xm = sb.tile([POS, P], F32, tag='xm')
    nc.vector.tensor_tensor(out=xm[:], in0=hacc[0:POS, 2 * P:3 * P],
                            in1=rb3[:], op=OP.mult)
    nc.vector.memset(combT[FPAD:D, :], 0.0)
    nc.vector.tensor_tensor(out=combT[FPAD:FPAD + POS, :], in0=xm[:],
                            in1=shTp[FPAD:FPAD + POS, :], op=OP.add)
    cmTp = ps.tile([P, D], F32, tag='ps', bufs=3, padded_shape=[P, 512])
    nc.tensor.transpose(out=cmTp[:], in_=combT[:],
                        identity=ident[0:D, 0:D])
    comb = sb.tile([P, D], F32, tag='comb')
    nc.scalar.copy(comb[:], cmTp[:])
    if li < 2:
        nc.sync.dma_start(out=shard_out[li][t * P:(t + 1) * P, :],
                          in_=comb[:])
    else:
        og = sb.tile([P, G], F32, tag='og')
        nc.vector.tensor_tensor(
            out=og[:], in0=batch_t[:, t:t + 1].to_broadcast([P, G]),
            in1=iotaG[:], op=OP.is_equal)
        pp = ps.tile([G, OUT + POS], F32, tag='psT', bufs=2, padded_shape=[P, 512])
        nc.tensor.matmul(pp[:], lhsT=og[:], rhs=comb[:, 0:OUT + POS],
                         start=True, stop=True)
        nc.vector.tensor_tensor(out=pool_acc[:], in0=pool_acc[:], in1=pp[:],
                                op=OP.add)


# ------------------------------------------------------------------- driver
_CACHE = {}


def get_compiled(plan):
    key = (plan['N'], plan['F0'], plan['CH'], plan['blocks'])
    if key not in _CACHE:
        _CACHE[key] = _build(plan)
    return _CACHE[key]


def prepare(feat, coord, edge_attr, edge_index, batch, params):
    plan, shared, core_inputs, head = _prep(feat, coord, edge_attr,
                                            edge_index, batch, params)
    nc = get_compiled(plan)
    in_maps = []
    for c in range(NCORES):
        m = dict(shared)
        m.update(core_inputs[c])
        in_maps.append(m)
    return nc, in_maps, head


def finish(results, head):
    pooled = np.zeros((G, OUT + POS), np.float32)
    for r in results:
        pooled += r['pooled']
    hw1, hb1, hw2, hb2 = head
    h = np.maximum(pooled @ hw1 + hb1, 0.0)
    return (h @ hw2 + hb2).astype(np.float32)


def kernel(feat, coord, edge_attr, edge_index, batch, params):
    nc, in_maps, head = prepare(feat, coord, edge_attr, edge_index, batch,
                                params)
    res = run_bass_kernel_spmd(nc, in_maps, core_ids=list(range(NCORES)))
    return finish(res.results, head)
